# revision 1
# baseline (speedup 1.0000x reference)
"""GPT-NeoX attention (B=4, S=1024, D=2048, H=16) on 8 TRN2 NeuronCores.

Tensor-parallel over heads: 2 heads per core. Each core computes its slice
of the fused QKV projection, RoPE, causal attention, and writes the
transposed per-head output [hd, S]; the host concatenates heads.

All on-chip matmuls use float32r (full PE speed at free-dim>=256) with
fp32 PSUM accumulation. Layouts are chosen so no on-chip transposes are
needed:
  - x is fed transposed  xT[feature, token]
  - q,k are produced transposed  qT/kT[hd, token]  (RoPE applied in place)
  - v is produced natural  v[token, hd]  via a second projection pass
  - scores are computed transposed  sT[k_token, q_token]
  - out is produced transposed  oT[hd, q_token] = v.T @ expT
  - softmax sum over k = ones-vector matmul; normalization applied to oT
    via a K=1 broadcast matmul of the reciprocal row.
"""

import os

import numpy as np

import concourse.bass as bass
import concourse.tile as tile
from concourse import bacc, mybir

# Problem constants (contract: nn_GPTNeoXAttention, fixed shapes)
B, S, D = 4, 1024, 2048
H = 16
HD = 128  # head dim
NCORES = 8
HPC = H // NCORES  # heads per core
ROPE_BASE = 10000.0
T = B * S  # 4096 tokens
KC = D // 128  # 16 contraction chunks of the model dim
NSL = 512  # token-slice width for the qk projection
NHALF = S // NSL  # 2 slices per batch
QCH = S // 512  # q slices per sequence in attention
SCALE = 1.0 / float(np.sqrt(HD))

F32 = mybir.dt.float32
F32R = mybir.dt.float32r

_CACHE = {}


def _build_program():
    nc = bacc.Bacc(
        "TRN2", target_bir_lowering=False, debug=False, num_devices=NCORES
    )

    x_d = nc.dram_tensor("x", [128, KC, T], F32R, kind="ExternalInput")
    w_d = nc.dram_tensor("w", [128, KC, 6 * HD], F32R, kind="ExternalInput")
    bqk_d = nc.dram_tensor("bqk", [128, 4], F32, kind="ExternalInput")
    bv_d = nc.dram_tensor("bv", [128, 2 * HD], F32, kind="ExternalInput")
    cos_d = nc.dram_tensor("cosT", [128, S], F32, kind="ExternalInput")
    sin_d = nc.dram_tensor("sinS", [128, S], F32, kind="ExternalInput")
    mask_d = nc.dram_tensor("masks", [128, 4, 512], F32, kind="ExternalInput")
    rot_d = nc.dram_tensor("rotT", [128, 128], F32R, kind="ExternalInput")
    ones_d = nc.dram_tensor("ones", [128, 128], F32R, kind="ExternalInput")
    out_d = nc.dram_tensor("out", [HPC, HD, B, S], F32, kind="ExternalOutput")

    x_ap = x_d.ap()
    w_ap = w_d.ap()
    out_ap = out_d.ap()

    Exp = mybir.ActivationFunctionType.Exp
    Identity = mybir.ActivationFunctionType.Identity

    with tile.TileContext(nc) as tc:
        with (
            tc.tile_pool(name="singles", bufs=1) as singles,
            tc.tile_pool(name="xin", bufs=2) as xin_pool,
            tc.tile_pool(name="qk", bufs=6) as qk_pool,
            tc.tile_pool(name="vp", bufs=2) as v_pool,
            tc.tile_pool(name="expp", bufs=4) as exp_pool,
            tc.tile_pool(name="tmp", bufs=3) as tmp_pool,
            tc.tile_pool(name="outp", bufs=3) as out_pool,
            tc.tile_pool(name="rcp", bufs=2) as rcp_pool,
            tc.tile_pool(name="ps_mix", bufs=2, space="PSUM") as ps_mix,
            tc.tile_pool(name="ps_s", bufs=2, space="PSUM") as ps_s,
            tc.tile_pool(name="ps_o", bufs=2, space="PSUM") as ps_o,
            tc.tile_pool(name="ps_sum", bufs=2, space="PSUM") as ps_sum,
        ):
            w_sb = singles.tile([128, KC, 6 * HD], F32R)
            for kc in range(KC):
                nc.scalar.dma_start(out=w_sb[:, kc, :], in_=w_ap[:, kc, :])
            cos_sb = singles.tile([128, S], F32)
            nc.gpsimd.dma_start(out=cos_sb, in_=cos_d.ap())
            sin_sb = singles.tile([128, S], F32)
            nc.gpsimd.dma_start(out=sin_sb, in_=sin_d.ap())
            bqk_sb = singles.tile([128, 4], F32)
            nc.gpsimd.dma_start(out=bqk_sb, in_=bqk_d.ap())
            bv_sb = singles.tile([128, 2 * HD], F32)
            nc.gpsimd.dma_start(out=bv_sb, in_=bv_d.ap())
            mask_sb = singles.tile([128, 4, 512], F32)
            nc.gpsimd.dma_start(out=mask_sb, in_=mask_d.ap())
            rot_sb = singles.tile([128, 128], F32R)
            nc.gpsimd.dma_start(out=rot_sb, in_=rot_d.ap())
            # ones[128,128] lhsT: ones.T @ expT = sum over k, replicated
            # across all 128 output partitions (broadcast-ready layout)
            ones_sb = singles.tile([128, 128], F32R)
            nc.gpsimd.dma_start(out=ones_sb, in_=ones_d.ap())

            for b in range(B):
                # feature-major q/k tiles for this batch:
                # m=0: q head0, m=1: q head1, m=2: k head0, m=3: k head1
                qk_tiles = [
                    qk_pool.tile([128, S], F32R, tag="qkt", name=f"qkt_{b}_{i}")
                    for i in range(4)
                ]
                # natural-layout v for this batch: [token(128), chunk, 2*HD]
                v_sb = v_pool.tile([128, S // 128, 2 * HD], F32R)

                for half in range(NHALF):
                    t0 = b * S + half * NSL  # global token offset
                    xsb = xin_pool.tile([128, KC, NSL], F32R)
                    nc.sync.dma_start(out=xsb, in_=x_ap[:, :, t0 : t0 + NSL])

                    sl = slice(half * NSL, (half + 1) * NSL)
                    # ---- q/k projection (transposed out: [feature, token]) ----
                    for m in range(4):
                        ps = ps_mix.tile([128, NSL], F32, tag="ps")
                        for kc in range(KC):
                            nc.tensor.matmul(
                                ps,
                                w_sb[:, kc, m * 128 : (m + 1) * 128],
                                xsb[:, kc, :],
                                start=(kc == 0),
                                stop=(kc == KC - 1),
                            )
                        # bias add (per-partition scalar) on ACT, PSUM -> SBUF
                        qb = tmp_pool.tile([128, NSL], F32R, tag="qb")
                        nc.scalar.activation(
                            qb, ps, Identity, bias=bqk_sb[:, m : m + 1], scale=1.0
                        )
                        # RoPE: rotate_half via PE permutation matmul, then
                        # same-partition elementwise combine on DVE.
                        dst = qk_tiles[m][:, sl]
                        ps2 = ps_mix.tile([128, NSL], F32, tag="ps")
                        nc.tensor.matmul(
                            ps2,
                            rot_sb,
                            qb,
                            start=True,
                            stop=True,
                        )
                        tmp2 = tmp_pool.tile([128, NSL], F32, tag="tmp2")
                        nc.vector.tensor_mul(tmp2, ps2, sin_sb[:, sl])
                        nc.vector.tensor_mul(dst, qb, cos_sb[:, sl])
                        nc.vector.tensor_add(dst, dst, tmp2)

                    # ---- v projection (natural out: [token, feature]) ----
                    for t in range(NSL // 128):
                        psv = ps_mix.tile([128, 2 * HD], F32, tag="ps")
                        for kc in range(KC):
                            nc.tensor.matmul(
                                psv,
                                xsb[:, kc, t * 128 : (t + 1) * 128],
                                w_sb[:, kc, 4 * 128 : 6 * 128],
                                start=(kc == 0),
                                stop=(kc == KC - 1),
                            )
                        nc.vector.tensor_add(
                            v_sb[:, half * (NSL // 128) + t, :], psv, bv_sb
                        )

                # ---- attention for this batch ----
                for h in range(HPC):
                    qT = qk_tiles[h]
                    kT = qk_tiles[2 + h]
                    for qs in range(QCH):
                        nk = (qs * 512 + 512) // 128  # causal: k chunks needed
                        ps_out = ps_o.tile([128, 512], F32)
                        ps_sm = ps_sum.tile([128, 512], F32)
                        qsl = slice(qs * 512, (qs + 1) * 512)
                        for ki in range(nk):
                            pss = ps_s.tile([128, 512], F32, tag="sc")
                            nc.tensor.matmul(
                                pss,
                                kT[:, ki * 128 : (ki + 1) * 128],
                                qT[:, qsl],
                                start=True,
                                stop=True,
                            )
                            e = exp_pool.tile([128, 512], F32R, tag="e")
                            nc.scalar.activation(e, pss, Exp, scale=SCALE)
                            off = ki * 128 - qs * 512
                            if 0 <= off <= 384:
                                nc.vector.tensor_mul(
                                    e, e, mask_sb[:, off // 128, :]
                                )
                            nc.tensor.matmul(
                                ps_out,
                                v_sb[:, ki, h * HD : (h + 1) * HD],
                                e,
                                start=(ki == 0),
                                stop=(ki == nk - 1),
                            )
                            nc.tensor.matmul(
                                ps_sm,
                                ones_sb,
                                e,
                                start=(ki == 0),
                                stop=(ki == nk - 1),
                            )
                        rc = rcp_pool.tile([128, 512], F32)
                        nc.vector.reciprocal_approx_fast(out=rc, in_=ps_sm)
                        o = out_pool.tile([128, 512], F32)
                        nc.vector.tensor_mul(o, ps_out, rc)
                        nc.sync.dma_start(
                            out=out_ap[h, :, b, qsl], in_=o
                        )

    nc.compile()
    return nc


def _prep_shared(hidden_states):
    x2 = np.ascontiguousarray(hidden_states.reshape(T, D).T)  # [D, T]
    x_host = np.ascontiguousarray(
        x2.reshape(KC, 128, T).transpose(1, 0, 2)
    )  # [128, KC, T]

    inv = 1.0 / (ROPE_BASE ** (np.arange(0, HD, 2, dtype=np.float64) / HD))
    f = np.outer(inv, np.arange(S, dtype=np.float64))  # [64, S]
    cosT = np.concatenate([np.cos(f), np.cos(f)], axis=0).astype(np.float32)
    sinS = np.concatenate([np.sin(f), np.sin(f)], axis=0).astype(np.float32)

    p = np.arange(128)[:, None]
    fcol = np.arange(512)[None, :]
    masks = np.stack(
        [(fcol >= p + o).astype(np.float32) for o in (0, 128, 256, 384)], axis=1
    )  # [128, 4, 512]
    masks = np.ascontiguousarray(masks)

    # rotate_half as a matmul: out = lhsT.T @ rhs with lhsT = rotT gives
    # (R @ q)[i] = -q[i+64] (i<64), q[i-64] (i>=64)
    rotT = np.zeros((128, 128), np.float32)
    rotT[np.arange(64), np.arange(64) + 64] = 1.0
    rotT[np.arange(64) + 64, np.arange(64)] = -1.0
    return x_host, cosT, sinS, masks, rotT


def _core_rows(c):
    h0, h1 = 2 * c, 2 * c + 1
    rows = []
    for part in range(3):  # q, k, v blocks
        for h in (h0, h1):
            base = h * 3 * HD + part * HD
            rows.extend(range(base, base + HD))
    return np.asarray(rows)


def _prep_core(w_qkv, b_qkv, c):
    rows = _core_rows(c)
    wT = np.ascontiguousarray(w_qkv[rows, :].T)  # [D, 768]
    w_host = np.ascontiguousarray(
        wT.reshape(KC, 128, 6 * HD).transpose(1, 0, 2)
    )  # [128, KC, 768]
    b_sel = b_qkv[rows]
    bqk = np.ascontiguousarray(b_sel[: 4 * 128].reshape(4, 128).T)  # [128, 4]
    bv = np.ascontiguousarray(
        np.broadcast_to(b_sel[4 * 128 :], (128, 2 * HD))
    )  # [128, 256]
    return w_host, bqk, bv


def _make_in_maps(hidden_states, w_qkv, b_qkv):
    x_host, cosT, sinS, masks, rotT = _prep_shared(hidden_states)
    in_maps = []
    for c in range(NCORES):
        w_host, bqk, bv = _prep_core(w_qkv, b_qkv, c)
        in_maps.append(
            {
                "x": x_host,
                "w": w_host,
                "bqk": bqk,
                "bv": bv,
                "cosT": cosT,
                "sinS": sinS,
                "masks": masks,
                "rotT": rotT,
                "ones": np.ones((128, 128), np.float32),
            }
        )
    return in_maps


def _assemble(results):
    outs = np.stack([results[c]["out"] for c in range(NCORES)])
    # [NCORES, HPC, HD, B, S] -> [B, S, H*HD]
    return np.ascontiguousarray(
        outs.reshape(H, HD, B, S).transpose(2, 3, 0, 1).reshape(B, S, D)
    )


def run(hidden_states, w_qkv, b_qkv, trace=False):
    from concourse.bass_utils import run_bass_kernel_spmd

    if "nc" not in _CACHE:
        _CACHE["nc"] = _build_program()
    nc = _CACHE["nc"]
    in_maps = _make_in_maps(
        np.asarray(hidden_states, dtype=np.float32),
        np.asarray(w_qkv, dtype=np.float32),
        np.asarray(b_qkv, dtype=np.float32),
    )
    res = run_bass_kernel_spmd(
        nc, in_maps, core_ids=list(range(NCORES)), trace=trace
    )
    out = _assemble(res.results)
    return out, res


def kernel(hidden_states, w_qkv, b_qkv):
    trace = os.environ.get("KERNEL_TRACE", "0") == "1"
    out, _res = run(hidden_states, w_qkv, b_qkv, trace=trace)
    return out



# revision 6
# speedup vs baseline: 1.1524x; 1.1524x over previous
"""GPT-NeoX attention (B=4, S=1024, D=2048, H=16) on 8 TRN2 NeuronCores.

Tensor-parallel over heads: 2 heads per core. Each core computes its slice
of the fused QKV projection, RoPE, causal attention, and writes the
transposed per-head output [hd, S]; the host concatenates heads.

All on-chip matmuls use float32r (full PE speed at free-dim>=256) with
fp32 PSUM accumulation. Layouts avoid on-chip transposes:
  - x is fed transposed  xT[feature, token]
  - q,k are produced transposed  qT/kT[hd, token]  (RoPE applied in place)
  - v is produced natural  v[token, hd]  via x-stationary matmuls
  - scores are computed transposed  sT[k_token, q_token]
  - out is produced transposed  oT[hd, q_token] = v.T @ expT
  - softmax sum over k = ones-matmul; normalization applied to oT

Scheduling structure (vs the naive version):
  - x and w stream per-kc chunk on separate DMA queues so the first
    matmul waits for ~320KB, not the whole 10.5MB prefetch.
  - The projection loop is kc-outer with 4 open PSUM groups, so compute
    tracks chunk arrival during the DMA-paced first batch.
  - Attention runs on 256-wide q-chunks (finer causal trim) and is
    software-pipelined into the RoPE/bias windows of the next half.
"""

import os

import numpy as np

import concourse.bass as bass
import concourse.tile as tile
from concourse import bacc, mybir

# Problem constants (contract: nn_GPTNeoXAttention, fixed shapes)
B, S, D = 4, 1024, 2048
H = 16
HD = 128  # head dim
NCORES = 8
HPC = H // NCORES  # heads per core
ROPE_BASE = 10000.0
T = B * S  # 4096 tokens
KC = D // 128  # 16 contraction chunks of the model dim
NSL = 512  # token-slice width (half a sequence)
QW = 256  # attention q-chunk width
SCALE = 1.0 / float(np.sqrt(HD))

F32 = mybir.dt.float32
F32R = mybir.dt.float32r

_CACHE = {}


def _build_program():
    nc = bacc.Bacc(
        "TRN2", target_bir_lowering=False, debug=False, num_devices=NCORES
    )

    x_d = nc.dram_tensor("x", [128, KC, T], F32R, kind="ExternalInput")
    w_d = nc.dram_tensor("w", [128, KC, 6 * HD], F32R, kind="ExternalInput")
    bqk_d = nc.dram_tensor("bqk", [128, 4], F32, kind="ExternalInput")
    bv_d = nc.dram_tensor("bv", [128, 2 * HD], F32, kind="ExternalInput")
    cos_d = nc.dram_tensor("cosT", [128, S], F32, kind="ExternalInput")
    sin_d = nc.dram_tensor("sinS", [128, S], F32, kind="ExternalInput")
    mask_d = nc.dram_tensor("masks", [128, 2, QW], F32, kind="ExternalInput")
    rot_d = nc.dram_tensor("rotT", [128, 128], F32R, kind="ExternalInput")
    ones_d = nc.dram_tensor("ones", [128, 128], F32R, kind="ExternalInput")
    out_d = nc.dram_tensor("out", [HPC, HD, B, S], F32, kind="ExternalOutput")

    x_ap = x_d.ap()
    w_ap = w_d.ap()
    out_ap = out_d.ap()

    Exp = mybir.ActivationFunctionType.Exp
    Identity = mybir.ActivationFunctionType.Identity

    with tile.TileContext(nc) as tc:
        with (
            tc.tile_pool(name="singles", bufs=1) as singles,
            tc.tile_pool(name="xin", bufs=2) as xin_pool,
            tc.tile_pool(name="qk", bufs=8) as qk_pool,
            tc.tile_pool(name="vp", bufs=2) as v_pool,
            tc.tile_pool(name="qbp", bufs=3) as qb_pool,
            tc.tile_pool(name="t2p", bufs=2) as t2_pool,
            tc.tile_pool(name="expp", bufs=4) as exp_pool,
            tc.tile_pool(name="outp", bufs=3) as out_pool,
            tc.tile_pool(name="rcp", bufs=2) as rcp_pool,
            tc.tile_pool(name="ps_qk", bufs=2, space="PSUM") as ps_qk,
            tc.tile_pool(name="ps_v", bufs=2, space="PSUM") as ps_v,
            tc.tile_pool(name="ps_s", bufs=2, space="PSUM") as ps_s,
            tc.tile_pool(name="ps_o", bufs=1, space="PSUM") as ps_o,
            tc.tile_pool(name="ps_sum", bufs=1, space="PSUM") as ps_sum,
        ):
            # --- weights stream per-kc on the scalar queue ---
            w_sb = []
            for kc in range(KC):
                wt = singles.tile(
                    [128, 6 * HD], F32R, tag=f"w{kc}", name=f"w_sb_{kc}"
                )
                nc.scalar.dma_start(out=wt, in_=w_ap[:, kc, :])
                w_sb.append(wt)
            # --- small constants on the sync queue (idle at start) ---
            bqk_sb = singles.tile([128, 4], F32, tag="bqk")
            nc.sync.dma_start(out=bqk_sb, in_=bqk_d.ap())
            rot_sb = singles.tile([128, 128], F32R, tag="rot")
            nc.sync.dma_start(out=rot_sb, in_=rot_d.ap())
            cos_sb = singles.tile([128, S], F32, tag="cos")
            nc.sync.dma_start(out=cos_sb, in_=cos_d.ap())
            sin_sb = singles.tile([128, S], F32, tag="sin")
            nc.sync.dma_start(out=sin_sb, in_=sin_d.ap())
            bv_sb = singles.tile([128, 2 * HD], F32, tag="bv")
            nc.sync.dma_start(out=bv_sb, in_=bv_d.ap())
            mask_sb = singles.tile([128, 2, QW], F32, tag="mask")
            nc.sync.dma_start(out=mask_sb, in_=mask_d.ap())
            # ones[128,128] lhsT: ones.T @ expT = sum over k on all partitions
            ones_sb = singles.tile([128, 128], F32R, tag="ones")
            nc.sync.dma_start(out=ones_sb, in_=ones_d.ap())

            qk_tiles = {}  # (b, m) -> feature-major q/k tile [128, S]
            v_tiles = {}  # b -> natural v tile [128, 8, 2*HD]

            def issue_x(b, half):
                t0 = b * S + half * NSL
                xs = []
                for kc in range(KC):
                    xt = xin_pool.tile(
                        [128, NSL], F32R, tag=f"x{kc}", name=f"x_{b}_{half}_{kc}"
                    )
                    nc.gpsimd.dma_start(
                        out=xt, in_=x_ap[:, kc, t0 : t0 + NSL]
                    )
                    xs.append(xt)
                return xs

            def proj_sweep(b, half, xs, sweep):
                """q heads (sweep=0) or k heads (sweep=1): two qk PSUM
                groups accumulated kc-outer so compute tracks DMA arrival."""
                pqs = [
                    ps_qk.tile(
                        [128, NSL], F32, tag="qk", name=f"pq_{b}_{half}_{sweep}_{i}"
                    )
                    for i in range(2)
                ]
                for kc in range(KC):
                    for i in range(2):
                        m = 2 * sweep + i
                        nc.tensor.matmul(
                            pqs[i],
                            w_sb[kc][:, m * 128 : (m + 1) * 128],
                            xs[kc],
                            start=(kc == 0),
                            stop=(kc == KC - 1),
                        )
                return pqs

            def v_pass(b, half, xs, pair):
                """natural-layout v for token tiles (2*pair, 2*pair+1)."""
                for i in range(2):
                    t = 2 * pair + i
                    pv = ps_v.tile(
                        [128, 2 * HD], F32, tag="v", name=f"pv_{b}_{half}_{t}"
                    )
                    for kc in range(KC):
                        nc.tensor.matmul(
                            pv,
                            xs[kc][:, t * 128 : (t + 1) * 128],
                            w_sb[kc][:, 4 * 128 : 6 * 128],
                            start=(kc == 0),
                            stop=(kc == KC - 1),
                        )
                    nc.vector.tensor_add(
                        v_tiles[b][:, half * 4 + t, :], pv, bv_sb
                    )

            def rope_pair(b, half, sweep, pqs):
                """bias + RoPE for the two feature blocks of one sweep."""
                sl = slice(half * NSL, (half + 1) * NSL)
                for i in range(2):
                    m = 2 * sweep + i
                    qb = qb_pool.tile(
                        [128, NSL], F32R, tag="qb", name=f"qb_{b}_{half}_{m}"
                    )
                    nc.scalar.activation(
                        qb, pqs[i], Identity, bias=bqk_sb[:, m : m + 1], scale=1.0
                    )
                    ps2 = ps_qk.tile(
                        [128, NSL], F32, tag="qk", name=f"ps2_{b}_{half}_{m}"
                    )
                    nc.tensor.matmul(ps2, rot_sb, qb, start=True, stop=True)
                    dst = qk_tiles[(b, m)][:, sl]
                    tmp2 = t2_pool.tile(
                        [128, NSL], F32, tag="t2", name=f"t2_{b}_{half}_{m}"
                    )
                    nc.vector.tensor_mul(tmp2, ps2, sin_sb[:, sl])
                    nc.vector.tensor_mul(dst, qb, cos_sb[:, sl])
                    nc.vector.tensor_add(dst, dst, tmp2)

            def attn_qs(b, h, qs):
                """one 256-wide q-chunk of causal attention for head h."""
                nk = 2 * (qs + 1)
                qT = qk_tiles[(b, h)]
                kT = qk_tiles[(b, 2 + h)]
                qsl = slice(qs * QW, (qs + 1) * QW)
                ps_out = ps_o.tile([128, QW], F32, tag="po", name=f"po_{b}_{h}_{qs}")
                ps_sm = ps_sum.tile([128, QW], F32, tag="pm", name=f"pm_{b}_{h}_{qs}")
                for ki in range(nk):
                    pss = ps_s.tile(
                        [128, QW], F32, tag="sc", name=f"sc_{b}_{h}_{qs}_{ki}"
                    )
                    nc.tensor.matmul(
                        pss,
                        kT[:, ki * 128 : (ki + 1) * 128],
                        qT[:, qsl],
                        start=True,
                        stop=True,
                    )
                    e = exp_pool.tile(
                        [128, QW], F32R, tag="e", name=f"e_{b}_{h}_{qs}_{ki}"
                    )
                    nc.scalar.activation(e, pss, Exp, scale=SCALE)
                    off = ki * 128 - qs * QW
                    if 0 <= off <= 128:
                        nc.vector.tensor_mul(e, e, mask_sb[:, off // 128, :])
                    nc.tensor.matmul(
                        ps_out,
                        v_tiles[b][:, ki, h * HD : (h + 1) * HD],
                        e,
                        start=(ki == 0),
                        stop=(ki == nk - 1),
                    )
                    nc.tensor.matmul(
                        ps_sm,
                        ones_sb,
                        e,
                        start=(ki == 0),
                        stop=(ki == nk - 1),
                    )
                rc = rcp_pool.tile([128, QW], F32, tag="rc", name=f"rc_{b}_{h}_{qs}")
                nc.vector.reciprocal_approx_fast(out=rc, in_=ps_sm)
                o = out_pool.tile([128, QW], F32, tag="o", name=f"o_{b}_{h}_{qs}")
                nc.vector.tensor_mul(o, ps_out, rc)
                nc.sync.dma_start(out=out_ap[h, :, b, qsl], in_=o)

            for b in range(B):
                for m in range(4):
                    qk_tiles[(b, m)] = qk_pool.tile(
                        [128, S], F32R, tag="qkt", name=f"qkt_{b}_{m}"
                    )
                v_tiles[b] = v_pool.tile(
                    [128, S // 128, 2 * HD], F32R, tag="vt", name=f"v_{b}"
                )

                for half in range(2):
                    xs = issue_x(b, half)
                    pqs = proj_sweep(b, half, xs, 0)
                    # pipelined attention fills the PE while ACT/DVE run RoPE
                    if half == 0:
                        if b > 0:
                            for h in range(HPC):
                                attn_qs(b - 1, h, 2)
                    else:
                        for h in range(HPC):
                            attn_qs(b, h, 0)
                    rope_pair(b, half, 0, pqs)
                    v_pass(b, half, xs, 0)
                    pqs = proj_sweep(b, half, xs, 1)
                    if half == 0:
                        if b > 0:
                            for h in range(HPC):
                                attn_qs(b - 1, h, 3)
                    else:
                        for h in range(HPC):
                            attn_qs(b, h, 1)
                    rope_pair(b, half, 1, pqs)
                    v_pass(b, half, xs, 1)

            for h in range(HPC):
                for qs in (2, 3):
                    attn_qs(B - 1, h, qs)

    nc.compile()
    return nc


def _prep_shared(hidden_states):
    x2 = np.ascontiguousarray(hidden_states.reshape(T, D).T)  # [D, T]
    x_host = np.ascontiguousarray(
        x2.reshape(KC, 128, T).transpose(1, 0, 2)
    )  # [128, KC, T]

    inv = 1.0 / (ROPE_BASE ** (np.arange(0, HD, 2, dtype=np.float64) / HD))
    f = np.outer(inv, np.arange(S, dtype=np.float64))  # [64, S]
    cosT = np.concatenate([np.cos(f), np.cos(f)], axis=0).astype(np.float32)
    sinS = np.concatenate([np.sin(f), np.sin(f)], axis=0).astype(np.float32)

    p = np.arange(128)[:, None]
    fcol = np.arange(QW)[None, :]
    masks = np.stack(
        [(fcol >= p + o).astype(np.float32) for o in (0, 128)], axis=1
    )  # [128, 2, QW]
    masks = np.ascontiguousarray(masks)

    # rotate_half as a matmul: out = lhsT.T @ rhs with lhsT = rotT gives
    # (R @ q)[i] = -q[i+64] (i<64), q[i-64] (i>=64)
    rotT = np.zeros((128, 128), np.float32)
    rotT[np.arange(64), np.arange(64) + 64] = 1.0
    rotT[np.arange(64) + 64, np.arange(64)] = -1.0
    return x_host, cosT, sinS, masks, rotT


def _core_rows(c):
    h0, h1 = 2 * c, 2 * c + 1
    rows = []
    for part in range(3):  # q, k, v blocks
        for h in (h0, h1):
            base = h * 3 * HD + part * HD
            rows.extend(range(base, base + HD))
    return np.asarray(rows)


def _prep_core(w_qkv, b_qkv, c):
    rows = _core_rows(c)
    wT = np.ascontiguousarray(w_qkv[rows, :].T)  # [D, 768]
    w_host = np.ascontiguousarray(
        wT.reshape(KC, 128, 6 * HD).transpose(1, 0, 2)
    )  # [128, KC, 768]
    b_sel = b_qkv[rows]
    bqk = np.ascontiguousarray(b_sel[: 4 * 128].reshape(4, 128).T)  # [128, 4]
    bv = np.ascontiguousarray(
        np.broadcast_to(b_sel[4 * 128 :], (128, 2 * HD))
    )  # [128, 256]
    return w_host, bqk, bv


def _make_in_maps(hidden_states, w_qkv, b_qkv):
    x_host, cosT, sinS, masks, rotT = _prep_shared(hidden_states)
    in_maps = []
    for c in range(NCORES):
        w_host, bqk, bv = _prep_core(w_qkv, b_qkv, c)
        in_maps.append(
            {
                "x": x_host,
                "w": w_host,
                "bqk": bqk,
                "bv": bv,
                "cosT": cosT,
                "sinS": sinS,
                "masks": masks,
                "rotT": rotT,
                "ones": np.ones((128, 128), np.float32),
            }
        )
    return in_maps


def _assemble(results):
    outs = np.stack([results[c]["out"] for c in range(NCORES)])
    # [NCORES, HPC, HD, B, S] -> [B, S, H*HD]
    return np.ascontiguousarray(
        outs.reshape(H, HD, B, S).transpose(2, 3, 0, 1).reshape(B, S, D)
    )


def run(hidden_states, w_qkv, b_qkv, trace=False):
    from concourse.bass_utils import run_bass_kernel_spmd

    if "nc" not in _CACHE:
        _CACHE["nc"] = _build_program()
    nc = _CACHE["nc"]
    in_maps = _make_in_maps(
        np.asarray(hidden_states, dtype=np.float32),
        np.asarray(w_qkv, dtype=np.float32),
        np.asarray(b_qkv, dtype=np.float32),
    )
    res = run_bass_kernel_spmd(
        nc, in_maps, core_ids=list(range(NCORES)), trace=trace
    )
    out = _assemble(res.results)
    return out, res


def kernel(hidden_states, w_qkv, b_qkv):
    trace = os.environ.get("KERNEL_TRACE", "0") == "1"
    out, _res = run(hidden_states, w_qkv, b_qkv, trace=trace)
    return out
